# revision 1
# baseline (speedup 1.0000x reference)
"""DeformableAttention1D on 8 TRN2 NeuronCores via Bass/Tile.

Sharding: core c handles offset-group g=c//2 (64 of 256 channels, 2 of 8 heads)
and query-half qh=c%2 (512 of 1024 positions). Each core computes its group's
offsets/gather/CPB/attention independently; the final output projection is
computed as a partial (wo sliced by group) and summed on the host (the
"all-reduce" of the output projection).

Device-side numerics: fp32 everywhere except the CPB relative-position-bias
MLP and its broadcast, which use fp32r matmuls (1 cycle/column vs 4 for fp32).
The ACT engine is restricted to ONE table set (natural_log_exp_and_others:
Exp/Ln/Relu/Copy/Identity/Square) because runtime table swaps are broken in
this environment; tanh and erf(gelu) are composed from Exp + DVE ops.
"""
import os
import sys

sys.path.insert(0, "/opt/trn_rl_repo")

DEBUG = bool(os.environ.get("DEFORM_DEBUG"))

import numpy as np

import concourse.bacc as bacc
import concourse.bass as bass
import concourse.mybir as mybir
import concourse.tile as tile
import concourse.bass_utils as bass_utils

F32 = mybir.dt.float32
F32R = mybir.dt.float32r
I32 = mybir.dt.int32
U32 = mybir.dt.uint32
AF = mybir.ActivationFunctionType
ALU = mybir.AluOpType

# model dims (hardcoded per problem spec)
DIM = 256
N = 1024
G = 4
HEADS = 8
DH = 32
NDS = 256          # downsampled kv positions
QS = 512           # queries per core
DPG = 64           # channels per group
OFF_K = 6
DS = 4             # downsample stride
OFF_SCALE = 4.0
NCORES = 8

# A&S 7.1.26 erf coefficients (|err| <= 1.5e-7)
ERF_P = 0.3275911
ERF_A = [0.254829592, -0.284496736, 1.421413741, -1.453152027, 1.061405429]

_CACHED = {}


def _patch_act_tables():
    """Restrict activation-table selection to the single set that covers all
    ACT functions used by this kernel, so exactly one table load is emitted
    (runtime table swaps do not work in this environment)."""
    import concourse.hw_specs as hw_specs

    if getattr(bacc, "_deform_act_patch", False):
        return
    orig = hw_specs.get_activation_tables

    keep = "natural_log_exp_and_others"

    def patched(module_arch):
        tabs = orig(module_arch)
        keep_funcs = tabs[keep]
        out = {}
        for name, funcs in tabs.items():
            if name == keep:
                out[name] = funcs
            else:
                out[name] = funcs - keep_funcs
        return out

    bacc.get_activation_tables = patched
    bacc._deform_act_patch = True


def _erf_gelu(nc, sb, out_ap, x_ap, shape):
    """out = 0.5 * x * (1 + erf(x/sqrt(2))) via A&S 7.1.26 (no erf table).

    Writes (1 + erf(x/sqrt2)) * x  (WITHOUT the 0.5 -- folded into wproj).
    """
    P, Nf = shape
    sq = sb.tile([P, Nf], F32, name="gelu_sq", tag="gelu_sq")
    nc.scalar.activation(sq[:], x_ap, AF.Square)
    e = sb.tile([P, Nf], F32, name="gelu_e", tag="gelu_e")
    # e = exp(-x^2/2)
    nc.scalar.activation(e[:], sq[:], AF.Exp, scale=-0.5)
    ax = sb.tile([P, Nf], F32, name="gelu_ax", tag="gelu_ax")
    # |x|/sqrt(2) = max(x, -x) * (1/sqrt2): two steps
    nc.vector.scalar_tensor_tensor(ax[:], x_ap, -1.0, x_ap, ALU.mult, ALU.max)
    t = sb.tile([P, Nf], F32, name="gelu_t", tag="gelu_t")
    # t = 1 / (1 + p * |x| / sqrt2)
    nc.vector.tensor_scalar(t[:], ax[:], float(ERF_P / np.sqrt(2.0)), 1.0, ALU.mult, ALU.add)
    nc.vector.reciprocal(t[:], t[:])
    poly = sb.tile([P, Nf], F32, name="gelu_poly", tag="gelu_poly")
    # P(t) = a1 t + a2 t^2 + ... + a5 t^5 via (x + c)*t nested form
    nc.vector.tensor_scalar(poly[:], t[:], ERF_A[4], ERF_A[3], ALU.mult, ALU.add)
    nc.vector.tensor_tensor(poly[:], poly[:], t[:], ALU.mult)
    nc.vector.scalar_tensor_tensor(poly[:], poly[:], ERF_A[2], t[:], ALU.add, ALU.mult)
    nc.vector.scalar_tensor_tensor(poly[:], poly[:], ERF_A[1], t[:], ALU.add, ALU.mult)
    nc.vector.scalar_tensor_tensor(poly[:], poly[:], ERF_A[0], t[:], ALU.add, ALU.mult)
    # poly*e = 1 - erf(|x|/sqrt2)  =>  erfa = 1 - poly*e
    erfa = sb.tile([P, Nf], F32, name="gelu_erfa", tag="gelu_erfa")
    nc.vector.tensor_tensor(erfa[:], poly[:], e[:], ALU.mult)
    nc.vector.tensor_scalar(erfa[:], erfa[:], -1.0, 1.0, ALU.mult, ALU.add)
    # copysign: erf(x) = sign(x)*erfa
    sgn = sb.tile([P, Nf], U32, name="gelu_sgn", tag="gelu_sgn")
    nc.vector.tensor_scalar(sgn[:], x_ap.bitcast(U32), 0x80000000, None, ALU.bitwise_and)
    erfs = sb.tile([P, Nf], F32, name="gelu_erfs", tag="gelu_erfs")
    nc.vector.tensor_tensor(erfs[:].bitcast(U32), erfa[:].bitcast(U32), sgn[:], ALU.bitwise_or)
    # out = (1 + erf) * x    (0.5 folded into wproj)
    nc.vector.tensor_scalar(erfs[:], erfs[:], 1.0, None, ALU.add)
    nc.vector.tensor_tensor(out_ap, erfs[:], x_ap, ALU.mult)


def _tanh_rows(nc, sb, out_ap, x_ap, shape):
    """out = tanh(x) = sign(x) * (1 - 2/(exp(2*min(|x|,30))+1)) on small tiles."""
    P, Nf = shape
    ax = sb.tile([P, Nf], F32, name="th_ax", tag="th_ax")
    nc.vector.scalar_tensor_tensor(ax[:], x_ap, -1.0, x_ap, ALU.mult, ALU.max)
    nc.vector.tensor_scalar(ax[:], ax[:], 30.0, None, ALU.min)
    e = sb.tile([P, Nf], F32, name="th_e", tag="th_e")
    nc.scalar.activation(e[:], ax[:], AF.Exp, scale=2.0)
    nc.vector.tensor_scalar(e[:], e[:], 1.0, None, ALU.add)
    r = sb.tile([P, Nf], F32, name="th_r", tag="th_r")
    nc.vector.reciprocal(r[:], e[:])
    # tha = 1 - 2r
    nc.vector.tensor_scalar(r[:], r[:], -2.0, 1.0, ALU.mult, ALU.add)
    sgn = sb.tile([P, Nf], U32, name="th_sgn", tag="th_sgn")
    nc.vector.tensor_scalar(sgn[:], x_ap.bitcast(U32), 0x80000000, None, ALU.bitwise_and)
    nc.vector.tensor_tensor(out_ap.bitcast(U32), r[:].bitcast(U32), sgn[:], ALU.bitwise_or)


def build_nc():
    _patch_act_tables()
    nc = bacc.Bacc("TRN2", target_bir_lowering=False, debug=False, num_devices=NCORES)

    # ---- per-core DRAM inputs ----
    din = {}

    def dt_in(name, shape):
        din[name] = nc.dram_tensor(name, shape, F32, kind="ExternalInput")
        return din[name]

    dt_in("xg", [DPG, N])
    dt_in("xq", [DPG, QS])
    dt_in("mask_st", [128, 32 * 128])
    # all small weights + identity packed into one tensor (one DMA)
    dt_in("packed", [128, 788])
    y_out = nc.dram_tensor("y", [DIM, QS], F32, kind="ExternalOutput")
    dbg = {}
    if DEBUG:
        for nm, shp in [("dbg_q", [DPG, N]), ("dbg_vgsp1", [1, NDS]),
                        ("dbg_rows4", [1, 4 * NDS]), ("dbg_kv", [DPG, NDS]),
                        ("dbg_k", [DPG, NDS]), ("dbg_v", [DPG, NDS]),
                        ("dbg_qs", [DPG, QS]), ("dbg_t0", [128, QS]),
                        ("dbg_bstk0", [128, NDS]), ("dbg_logit00", [128, QS]),
                        ("dbg_avn", [DPG, QS])]:
            dbg[nm] = nc.dram_tensor(nm, shp, F32, kind="ExternalOutput")

    NT = N // 128          # 8 n-tiles for gather
    NITER = QS // 2        # 256 CPB iterations (2 queries each)
    NSTACK = NITER // 32   # 8 bias stacks

    with tile.TileContext(nc) as tc:
        with (
            tc.tile_pool(name="const", bufs=1) as cst,
            tc.tile_pool(name="work", bufs=2) as wk,
            tc.tile_pool(name="rows", bufs=1) as rw,
            tc.tile_pool(name="persist", bufs=1) as pe_pool,
            tc.tile_pool(name="h1p", bufs=4) as h1p,
            tc.tile_pool(name="h2p", bufs=4) as h2p,
        ):
            # ---- load inputs: xg, xq, then one packed-weights DMA ----
            xg = cst.tile([DPG, N], F32, name="xg", tag="xg")
            nc.sync.dma_start(xg[:], din["xg"].ap())
            xq = cst.tile([DPG, QS], F32, name="xq", tag="xq")
            nc.sync.dma_start(xq[:], din["xq"].ap())
            packed = cst.tile([128, 788], F32, name="packed", tag="packed")
            nc.sync.dma_start(packed[:], din["packed"].ap())
            w2bd = packed[:, 0:128]
            eyet = packed[:, 128:256]
            wqT = packed[0:DPG, 256:320]
            wqTs = packed[0:DPG, 320:384]
            wkT = packed[0:DPG, 384:448]
            wvT = packed[0:DPG, 448:512]
            woT = packed[0:DPG, 512:768]
            wdw = packed[0:DPG, 768:774]
            bodw = packed[0:DPG, 774:775]
            wproj_half = packed[0:DPG, 775:776]
            b1col = packed[:, 776:777]
            b2col = packed[:, 777:778]
            b3bc = packed[:, 778:780]
            qbase = packed[:, 780:781]
            w3bd = packed[:, 781:785]

            ones_col = cst.tile([128, 1], F32, name="ones", tag="ones")
            nc.gpsimd.memset(ones_col[:], 1.0)
            # dummy activation: triggers the (single) ACT table load at t=0 so
            # it overlaps the input DMAs instead of sitting in the offsets chain
            warm = cst.tile([128, 1], F32, name="warm", tag="warm")
            nc.scalar.activation(warm[:], ones_col[:], AF.Relu)
            ones_colr = cst.tile([128, 1], F32R, name="onesr", tag="onesr")
            nc.vector.tensor_copy(ones_colr[:], ones_col[:])

            # fp32r copies of CPB weights
            w2bdr = cst.tile([128, 128], F32R, name="w2bdr", tag="w2bdr")
            nc.vector.tensor_copy(w2bdr[:], w2bd)
            w3bdr = cst.tile([128, 4], F32R, name="w3bdr", tag="w3bdr")
            nc.vector.tensor_copy(w3bdr[:], w3bd)
            woTr = cst.tile([DPG, DIM], F32R, name="woTr", tag="woTr")
            nc.vector.tensor_copy(woTr[:], woT)


            # persistent SBUF tiles that cross phase boundaries
            k_sb = pe_pool.tile([DPG, NDS], F32R, name="k_sb", tag="k_sb")
            qs_sb = pe_pool.tile([DPG, QS], F32R, name="qs_sb", tag="qs_sb")
            vT = [pe_pool.tile([128, DPG], F32R, name=f"vT{H}", tag=f"vT{H}") for H in range(2)]
            tT = [pe_pool.tile([128, NDS], F32R, name=f"tT{t}", tag=f"tT{t}") for t in range(4)]
            # bias, transposed into attention layout, packed as
            # col = ((((itq*2 + itl)*32 + pp)*2 + h1)*2 + c)*2 + o  (j on partitions)
            biasT_sb = pe_pool.tile([128, 4 * QS], F32, name="biasT_sb", tag="biasT_sb")
            avn = pe_pool.tile([DPG, QS], F32R, name="avn", tag="avn")

            # ============ phases A-C: q, offsets, gather, kv, t ============
            with tc.tile_pool(name="psA", bufs=2, space="PSUM") as psA:
                # ---- phase A: q + offsets ----
                q_pad = pe_pool.tile([DPG, N + 2], F32, name="q_pad", tag="q_pad")
                nc.gpsimd.memset(q_pad[:], 0.0)
                for h in range(2):
                    pq = psA.tile([DPG, QS], F32, name="pA512", tag="pA512")
                    nc.tensor.matmul(pq[:], wqT, xg[:, h * QS:(h + 1) * QS])
                    nc.scalar.copy(q_pad[:, 1 + h * QS:1 + (h + 1) * QS], pq[:])

                # depthwise strided conv (6 taps)
                acc = wk.tile([DPG, NDS], F32, name="conv_acc", tag="conv_acc")
                nc.vector.tensor_scalar(
                    acc[:], q_pad[:, 0:N - 3:DS], wdw[:, 0:1], bodw, ALU.mult, ALU.add)
                for kk in range(1, OFF_K):
                    nc.vector.scalar_tensor_tensor(
                        acc[:], q_pad[:, kk:kk + N - 3:DS], wdw[:, kk:kk + 1], acc[:],
                        ALU.mult, ALU.add)

                if DEBUG:
                    nc.sync.dma_start(dbg["dbg_q"].ap(), q_pad[:, 1:N + 1])
                gl = wk.tile([DPG, NDS], F32, name="gelu_out", tag="gelu_out")
                _erf_gelu(nc, wk, gl[:], acc[:], [DPG, NDS])

                # proj row: [1, NDS] = sum_c 0.5*wproj[c] * gl[c, :]
                pproj = psA.tile([1, NDS], F32, name="pproj", tag="pproj")
                nc.tensor.matmul(pproj[:], wproj_half, gl[:])
                proj_sb = rw.tile([1, NDS], F32, name="proj_sb", tag="proj_sb")
                nc.vector.tensor_copy(proj_sb[:], pproj[:])
                th = rw.tile([1, NDS], F32, name="th", tag="th")
                _tanh_rows(nc, rw, th[:], proj_sb[:], [1, NDS])

                # vgrid = j + 4*tanh ; vgsp1 = vgrid*2/255 ; p_pix = vgsp1*512 - 0.5
                iotaj = rw.tile([1, NDS], I32, name="iotaj", tag="iotaj")
                nc.gpsimd.iota(iotaj[:], pattern=[[1, NDS]], base=0, channel_multiplier=0)
                iotajf = rw.tile([1, NDS], F32, name="iotajf", tag="iotajf")
                nc.vector.tensor_copy(iotajf[:], iotaj[:])
                vgrid = rw.tile([1, NDS], F32, name="vgrid", tag="vgrid")
                nc.vector.scalar_tensor_tensor(vgrid[:], th[:], OFF_SCALE, iotajf[:], ALU.mult, ALU.add)
                vgsp1 = rw.tile([1, NDS], F32, name="vgsp1", tag="vgsp1")
                nc.vector.tensor_scalar(vgsp1[:], vgrid[:], float(2.0 / (NDS - 1)), None, ALU.mult)
                ppix = rw.tile([1, NDS], F32, name="ppix", tag="ppix")
                nc.vector.tensor_scalar(ppix[:], vgsp1[:], float(N / 2.0), -0.5, ALU.mult, ALU.add)

                # rows4 = [i0f | i1f | w0 | w1]
                rows4 = rw.tile([1, 4 * NDS], F32, name="rows4", tag="rows4")
                i0i = rw.tile([1, NDS], I32, name="i0i", tag="i0i")
                nc.vector.tensor_copy(i0i[:], ppix[:])
                i0c = rw.tile([1, NDS], F32, name="i0c", tag="i0c")
                nc.vector.tensor_copy(i0c[:], i0i[:])
                # floor(p) regardless of the convert rounding mode:
                # i0 = cvt(p) - (cvt(p) > p)
                gt = rw.tile([1, NDS], F32, name="gt", tag="gt")
                nc.vector.tensor_tensor(gt[:], i0c[:], ppix[:], ALU.is_gt)
                nc.vector.tensor_tensor(rows4[:, 0:NDS], i0c[:], gt[:], ALU.subtract)
                nc.vector.tensor_scalar(rows4[:, NDS:2 * NDS], rows4[:, 0:NDS], 1.0, None, ALU.add)
                nc.vector.tensor_tensor(rows4[:, 3 * NDS:4 * NDS], ppix[:], rows4[:, 0:NDS], ALU.subtract)
                nc.vector.tensor_scalar(rows4[:, 2 * NDS:3 * NDS], rows4[:, 3 * NDS:4 * NDS], -1.0, 1.0, ALU.mult, ALU.add)

                if DEBUG:
                    nc.sync.dma_start(dbg["dbg_vgsp1"].ap(), vgsp1[:])
                    nc.sync.dma_start(dbg["dbg_rows4"].ap(), rows4[:])
                bc4 = pe_pool.tile([128, 4 * NDS], F32, name="bc4", tag="bc4")
                nc.gpsimd.partition_broadcast(bc4[:], rows4[:])

                # vgsp1 as per-partition columns for the two j-halves
                # (PE transpose of the row -- avoids DMA queue latency)
                vgsp1c = cst.tile([128, 2], F32, name="vgsp1c", tag="vgsp1c")
                for H in range(2):
                    ptv = psA.tile([128, 128], F32, name="ptv", tag="ptp")
                    nc.tensor.transpose(ptv[:, 0:1], vgsp1[:, H * 128:(H + 1) * 128],
                                        eyet[0:1, 0:1])
                    nc.vector.tensor_copy(vgsp1c[:, H:H + 1], ptv[:, 0:1])

                # CPB selection masks (one prepacked DMA + fp32r round)
                maskr = pe_pool.tile([128, 32 * 128], F32R, name="maskr", tag="maskr")
                with tc.tile_pool(name="maskst", bufs=1) as mp:
                    mask_st = mp.tile([128, 32 * 128], F32, name="mask_st", tag="mask_st")
                    nc.sync.dma_start(mask_st[:], din["mask_st"].ap())
                    nc.vector.tensor_copy(maskr[:], mask_st[:])

                # ---- phase C: t = sign(pos)*log1p(|pos|), transposed ----
                io = wk.tile([128, QS], I32, name="io", tag="io")
                nc.gpsimd.iota(io[:], pattern=[[1, QS]], base=0, channel_multiplier=0)
                gqp = wk.tile([128, QS], F32, name="gqp", tag="gqp")
                nc.vector.tensor_scalar(gqp[:], io[:], qbase, float(2.0 / (N - 1)), ALU.add, ALU.mult)

                for H in range(2):
                    pos = wk.tile([128, QS], F32, name="pos", tag="pos")
                    nc.vector.tensor_scalar(pos[:], gqp[:], vgsp1c[:, H:H + 1], None, ALU.subtract)
                    apos = wk.tile([128, QS], F32, name="apos", tag="apos")
                    nc.vector.scalar_tensor_tensor(apos[:], pos[:], -1.0, pos[:], ALU.mult, ALU.max)
                    aln = wk.tile([128, QS], F32, name="aln", tag="aln")
                    nc.scalar.activation(aln[:], apos[:], AF.Ln, bias=1.0)
                    sgn = wk.tile([128, QS], U32, name="psgn", tag="psgn")
                    nc.vector.tensor_scalar(sgn[:], pos[:].bitcast(U32), 0x80000000, None, ALU.bitwise_and)
                    t_H = wk.tile([128, QS], F32, name="t_H", tag="t_H")
                    nc.vector.tensor_tensor(t_H[:].bitcast(U32), aln[:].bitcast(U32), sgn[:], ALU.bitwise_or)
                    if DEBUG and H == 0:
                        nc.sync.dma_start(dbg["dbg_t0"].ap(), t_H[:])
                    for it in range(4):
                        ptp = psA.tile([128, 128], F32, name="ptp", tag="ptp")
                        nc.tensor.transpose(ptp[:], t_H[:, it * 128:(it + 1) * 128], eyet)
                        nc.scalar.copy(tT[it][:, H * 128:(H + 1) * 128], ptp[:])

                # selection masks for the CPB broadcast. Iteration p reads
                # tT rows (2p, 2p+1); those sit inside the 32-aligned window
                # [32*(p//16), +32), so a [32, 128] mask indexed by p%16
                # suffices (16 variants).

                # ---- phase B: gather (one-hot matmul), kv, k, v, vT ----
                xgT = []
                for t in range(NT):
                    pt = psA.tile([128, 128], F32, name="ptp", tag="ptp")
                    nc.tensor.transpose(pt[:, 0:DPG], xg[:, t * 128:(t + 1) * 128], eyet[0:DPG, 0:DPG])
                    st = pe_pool.tile([128, DPG], F32, name=f"xgT{t}", tag=f"xgT{t}")
                    nc.scalar.copy(st[:], pt[:, 0:DPG])
                    xgT.append(st)

                pkv = psA.tile([DPG, NDS], F32, name="pA256", tag="pA256")
                for t in range(NT):
                    icol = wk.tile([128, 1], I32, name="icol", tag="icol")
                    nc.gpsimd.iota(icol[:], pattern=[[0, 1]], base=t * 128, channel_multiplier=1)
                    icolf = wk.tile([128, 1], F32, name="icolf", tag="icolf")
                    nc.vector.tensor_copy(icolf[:], icol[:])
                    eq0 = wk.tile([128, NDS], F32, name="eq0", tag="eq0")
                    nc.vector.tensor_scalar(eq0[:], bc4[:, 0:NDS], icolf[:], None, ALU.is_equal)
                    s0 = wk.tile([128, NDS], F32, name="s0", tag="s0")
                    nc.vector.tensor_tensor(s0[:], eq0[:], bc4[:, 2 * NDS:3 * NDS], ALU.mult)
                    eq1 = wk.tile([128, NDS], F32, name="eq1", tag="eq1")
                    nc.vector.tensor_scalar(eq1[:], bc4[:, NDS:2 * NDS], icolf[:], None, ALU.is_equal)
                    s1 = wk.tile([128, NDS], F32, name="s1", tag="s1")
                    nc.vector.tensor_tensor(s1[:], eq1[:], bc4[:, 3 * NDS:4 * NDS], ALU.mult)
                    S = wk.tile([128, NDS], F32, name="S", tag="S")
                    nc.vector.tensor_tensor(S[:], s0[:], s1[:], ALU.add)
                    nc.tensor.matmul(pkv[:], xgT[t][:], S[:],
                                     start=(t == 0), stop=(t == NT - 1))
                kv = wk.tile([DPG, NDS], F32, name="kv", tag="kv")
                nc.scalar.copy(kv[:], pkv[:])
                if DEBUG:
                    nc.sync.dma_start(dbg["dbg_kv"].ap(), kv[:])

                pk = psA.tile([DPG, NDS], F32, name="pA256", tag="pA256")
                nc.tensor.matmul(pk[:], wkT, kv[:])
                nc.scalar.copy(k_sb[:], pk[:])
                pv = psA.tile([DPG, NDS], F32, name="pA256", tag="pA256")
                nc.tensor.matmul(pv[:], wvT, kv[:])
                v_sb = wk.tile([DPG, NDS], F32, name="v_sb", tag="v_sb")
                nc.scalar.copy(v_sb[:], pv[:])
                if DEBUG:
                    nc.sync.dma_start(dbg["dbg_k"].ap(), k_sb[:])
                    nc.sync.dma_start(dbg["dbg_v"].ap(), v_sb[:])

                for H in range(2):
                    pt = psA.tile([128, 128], F32, name="ptp", tag="ptp")
                    nc.tensor.transpose(pt[:, 0:DPG], v_sb[:, H * 128:(H + 1) * 128], eyet[0:DPG, 0:DPG])
                    nc.scalar.copy(vT[H][:], pt[:, 0:DPG])

                # q_s (scaled q for this core's query half)
                pqs = psA.tile([DPG, QS], F32, name="pA512", tag="pA512")
                nc.tensor.matmul(pqs[:], wqTs, xq[:])
                nc.scalar.copy(qs_sb[:], pqs[:])
                if DEBUG:
                    nc.sync.dma_start(dbg["dbg_qs"].ap(), qs_sb[:])


            # ============ phase D: CPB MLP (fp32r) ============
            with (
                tc.tile_pool(name="ps1", bufs=4, space="PSUM") as ps1,
                tc.tile_pool(name="ps2", bufs=2, space="PSUM") as ps2,
                tc.tile_pool(name="ps3", bufs=1, space="PSUM") as ps3,
            ):
                # two psum banks collect the transposed bias (one per j-half),
                # spilled to SBUF when full (after every 64 pairs)
                biasTp = [ps3.tile([128, 2 * NDS], F32, name=f"biasTp{i}", tag=f"biasTp{i}")
                          for i in range(2)]
                for it in range(4):
                    for pp in range(32):  # two queries... pair-iteration
                        kpair = it * 32 + pp
                        pre1 = ps1.tile([128, 2 * NDS], F32, name="pre1", tag="pre1")
                        h1 = h1p.tile([128, 2 * NDS], F32R, name="h1", tag="h1")
                        pre2 = ps2.tile([128, 2 * NDS], F32, name="pre2", tag="pre2")
                        h2 = h2p.tile([128, 2 * NDS], F32R, name="h2", tag="h2")
                        for half in range(2):
                            p = 2 * pp + half
                            sl = slice(half * NDS, (half + 1) * NDS)
                            a, m = p // 32, p % 32
                            nc.tensor.matmul(
                                pre1[:, sl],
                                maskr[64 * a:64 * (a + 1), 128 * m:128 * (m + 1)],
                                tT[it][64 * a:64 * (a + 1), :])
                        r1_act = kpair % 2 == 1
                        r2_act = kpair % 2 == 0
                        if r1_act:
                            nc.scalar.activation(h1[:], pre1[:], AF.Relu, bias=b1col)
                        else:
                            nc.vector.tensor_scalar(h1[:], pre1[:], b1col, 0.0, ALU.add, ALU.max)
                        for half in range(2):
                            sl = slice(half * NDS, (half + 1) * NDS)
                            nc.tensor.matmul(pre2[:, sl], w2bdr[:], h1[:, sl])
                        if r2_act:
                            nc.scalar.activation(h2[:], pre2[:], AF.Relu, bias=b2col)
                        else:
                            nc.vector.tensor_scalar(h2[:], pre2[:], b2col, 0.0, ALU.add, ALU.max)
                        for h1f in range(2):
                            for H in range(2):
                                outn = 2 * kpair + h1f
                                slot = outn % 128
                                nc.tensor.matmul(
                                    biasTp[H][:, 4 * slot:4 * slot + 4],
                                    h2[:, 256 * h1f + 128 * H:256 * h1f + 128 * H + 128],
                                    w3bdr[:])
                        if kpair % 64 == 63:
                            bank = kpair // 64
                            nc.vector.tensor_copy(
                                biasT_sb[:, QS * bank:QS * (bank + 1)],
                                biasTp[0][:])
                            nc.scalar.copy(
                                biasT_sb[:, QS * (2 + bank):QS * (2 + bank + 1)],
                                biasTp[1][:])
                            if bank == 0:
                                biasTp = [ps3.tile([128, 2 * NDS], F32,
                                                   name=f"biasTp{i}b", tag=f"biasTp{i}")
                                          for i in range(2)]

            if DEBUG:
                nc.sync.dma_start(dbg["dbg_bstk0"].ap(), biasT_sb[:, 0:NDS])
            # ============ phase E: attention ============
            with (
                tc.tile_pool(name="psE", bufs=2, space="PSUM") as psE,
                tc.tile_pool(name="psE1", bufs=1, space="PSUM") as psE1,
            ):
                # biasT_sb column decomposition:
                # col = 2048*H? no: region (2H+bank)*QS, inner 4*slot + 2c + o
                # with slot = (2*(32it+pp) + h1) % 128 and bank = itq = it//2.
                # As i_loc = 128it + 4pp + 2h1 + c runs over [128it, 128it+128),
                # (pp, h1, c) iterate with c innermost -- matching free order.
                bview = biasT_sb[:].rearrange(
                    "p (r itl pp h1 c o) -> p r itl pp h1 c o",
                    r=4, itl=2, pp=32, h1=2, c=2, o=2)

                for h in range(2):
                    expT = []
                    for H in range(2):
                        psim = psE.tile([128, QS], F32, name="psim", tag="psim")
                        nc.tensor.matmul(
                            psim[:], k_sb[32 * h:32 * (h + 1), H * 128:(H + 1) * 128],
                            qs_sb[32 * h:32 * (h + 1), :])
                        logit = wk.tile([128, QS], F32, name="logit", tag="logit")
                        for it in range(4):
                            itq, itl = it // 2, it % 2
                            nc.vector.scalar_tensor_tensor(
                                logit[:, 128 * it:128 * (it + 1)],
                                bview[:, 2 * H + itq, itl, :, :, :, h],
                                b3bc[:, h:h + 1],
                                psim[:, 128 * it:128 * (it + 1)],
                                ALU.add, ALU.add)
                        if DEBUG and h == 0 and H == 0:
                            nc.sync.dma_start(dbg["dbg_logit00"].ap(), logit[:])
                        et = wk.tile([128, QS], F32R, name="expT", tag="expT")
                        nc.scalar.activation(et[:], logit[:], AF.Exp)
                        expT.append(et)

                    # sums over j via ones-matmul, then reciprocal
                    psum_s = psE1.tile([1, QS], F32, name="psum_s", tag="psum_s")
                    for H in range(2):
                        nc.tensor.matmul(psum_s[:], ones_colr[:], expT[H][:],
                                         start=(H == 0), stop=(H == 1))
                    rs = rw.tile([1, QS], F32, name="rs", tag="rs")
                    nc.vector.reciprocal(rs[:], psum_s[:])
                    rsb = wk.tile([32, QS], F32, name="rsb", tag="rsb")
                    nc.gpsimd.partition_broadcast(rsb[:], rs[:])

                    pav = psE1.tile([32, QS], F32, name="pav", tag="pav")
                    for H in range(2):
                        nc.tensor.matmul(pav[:], vT[H][:, 32 * h:32 * (h + 1)], expT[H][:],
                                         start=(H == 0), stop=(H == 1))
                    nc.vector.tensor_tensor(avn[32 * h:32 * (h + 1), :], pav[:], rsb[:], ALU.mult)

                for m in range(2):
                    py = psE.tile([128, QS], F32, name="py", tag="py")
                    nc.tensor.matmul(py[:], woTr[:, m * 128:(m + 1) * 128], avn[:])
                    y_sb = wk.tile([128, QS], F32, name="y_sb", tag="y_sb")
                    nc.scalar.copy(y_sb[:], py[:])
                    nc.sync.dma_start(y_out.ap()[m * 128:(m + 1) * 128, :], y_sb[:])

    nc.compile()
    return nc


def _shard_inputs(inputs):
    """Build the 8 per-core input maps from the full inputs."""
    x = np.ascontiguousarray(inputs["x"][0])              # [256, 1024]
    wq, wk, wv = inputs["wq"], inputs["wk"], inputs["wv"]  # [4, 64, 64]
    wo = inputs["wo"]                                      # [256, 256]
    w_off_dw = inputs["w_off_dw"][:, 0, :]                 # [64, 6]
    b_off_dw = inputs["b_off_dw"]                          # [64]
    w_off_proj = inputs["w_off_proj"]                      # [64]
    w1 = inputs["cpb_w1"][:, 0]                            # [64]
    b1 = inputs["cpb_b1"]                                  # [64]
    w2 = inputs["cpb_w2"]                                  # [64, 64]
    b2 = inputs["cpb_b2"]                                  # [64]
    w3 = inputs["cpb_w3"]                                  # [2, 64]
    b3 = inputs["cpb_b3"]                                  # [2]

    f = np.float32
    w1sel = np.zeros((2, 128), f)
    w1sel[0, :64] = w1
    w1sel[1, 64:] = w1
    mask_st = np.zeros((128, 32 * 128), f)
    for band in range(2):
        for m in range(32):
            mask_st[64 * band + 2 * m:64 * band + 2 * m + 2, 128 * m:128 * (m + 1)] = w1sel
    b1col = np.concatenate([b1, b1]).astype(f)[:, None]
    w2bd = np.zeros((128, 128), f)
    w2bd[:64, :64] = w2.T
    w2bd[64:, 64:] = w2.T
    b2col = np.concatenate([b2, b2]).astype(f)[:, None]
    w3bd = np.zeros((128, 4), f)
    w3bd[:64, :2] = w3.T
    w3bd[64:, 2:] = w3.T
    b3bc = np.broadcast_to(b3.astype(f)[None, :], (128, 2)).copy()
    base_packed = np.zeros((128, 788), f)
    base_packed[:, 0:128] = w2bd
    base_packed[:, 128:256] = np.eye(128, dtype=f)
    base_packed[:, 776:777] = b1col
    base_packed[:, 777:778] = b2col
    base_packed[:, 778:780] = b3bc
    base_packed[:, 781:785] = w3bd

    in_maps = []
    for c in range(NCORES):
        g, qh = c // 2, c % 2
        xg = np.ascontiguousarray(x[64 * g:64 * (g + 1)], dtype=f)
        pk = base_packed.copy()
        pk[0:64, 256:320] = wq[g].T
        pk[0:64, 320:384] = wq[g].T * f(DH) ** f(-0.5)
        pk[0:64, 384:448] = wk[g].T
        pk[0:64, 448:512] = wv[g].T
        pk[0:64, 512:768] = wo[:, 64 * g:64 * (g + 1)].T
        pk[0:64, 768:774] = w_off_dw
        pk[0:64, 774] = b_off_dw
        pk[0:64, 775] = 0.5 * w_off_proj
        pk[:, 780] = f(QS * qh)
        m = {
            "xg": xg,
            "xq": np.ascontiguousarray(xg[:, QS * qh:QS * (qh + 1)]),
            "mask_st": mask_st,
            "packed": pk,
        }
        in_maps.append(m)
    return in_maps


def kernel(**inputs):
    if "nc" not in _CACHED:
        _CACHED["nc"] = build_nc()
    nc = _CACHED["nc"]
    in_maps = _shard_inputs(inputs)
    res = bass_utils.run_bass_kernel_spmd(nc, in_maps, core_ids=list(range(NCORES)))
    ys = [res.results[c]["y"] for c in range(NCORES)]
    bo = inputs["bo"]
    out = np.zeros((1, DIM, N), np.float32)
    for qh in range(2):
        acc = np.zeros((DIM, QS), np.float64)
        for g in range(G):
            acc += ys[2 * g + qh]
        out[0, :, QS * qh:QS * (qh + 1)] = (acc + bo.astype(np.float64)[:, None]).astype(np.float32)
    return out



# revision 4
# speedup vs baseline: 3.3207x; 3.3207x over previous
"""DeformableAttention1D on 8 TRN2 NeuronCores via Bass/Tile.

Sharding: core c handles offset-group g=c//2 (64 of 256 channels, 2 of 8 heads)
and query-half qh=c%2 (512 of 1024 positions). Each core computes its group's
offsets/gather/bias/attention independently; the final output projection is
computed as a partial (wo sliced by group) and summed on the host (the
"all-reduce" of the output projection).

Key idea vs the straightforward implementation: the CPB relative-position-bias
MLP is a scalar->2 function F(d) of the signed distance d = gq_i - vgsp1_j,
and the query grid steps uniformly by h_q = 2/1023. Tabulating F on a grid
with spacing exactly h_q (a host-side weights-only precompute, like any other
weight repacking) turns the bias into

    bias[i, j] = (1-r_j) T[m_j + i] + r_j T[m_j + 1 + i],   m_j + r_j = c_j,

i.e. a per-column shifted window of the table. On device this is 9 matmuls of
"tent" interpolation one-hots (stationary) against host-precomputed Hankel
slabs of T (moving), accumulated directly into the attention-logit PSUM on
top of q.k — the bias costs zero vector-engine work in the attention phase.
The tent matrices max(0, 1-|c_j - kappa|) also implement the bilinear kv
grid-sample gather (zeros padding included), replacing is_equal one-hot
pairs. Max interp error in the logits is ~3e-4 (table spacing ~0.002).

Device numerics: fp32 data, fp32r matmuls (1 cycle/col vs 4 for fp32). The
ACT engine is restricted to ONE table set (natural_log_exp_and_others:
Exp/Ln/Relu/Copy/Identity/Square) because runtime table swaps are broken in
this environment; tanh and erf(gelu) are composed from Exp + DVE ops.
"""
import os
import sys

sys.path.insert(0, "/opt/trn_rl_repo")

DEBUG = bool(os.environ.get("DEFORM_DEBUG"))

import numpy as np

import concourse.bacc as bacc
import concourse.bass as bass
import concourse.mybir as mybir
import concourse.tile as tile
import concourse.bass_utils as bass_utils

F32 = mybir.dt.float32
F32R = mybir.dt.float32r
I32 = mybir.dt.int32
U32 = mybir.dt.uint32
AF = mybir.ActivationFunctionType
ALU = mybir.AluOpType

# model dims (hardcoded per problem spec)
DIM = 256
N = 1024
G = 4
HEADS = 8
DH = 32
NDS = 256          # downsampled kv positions
QS = 512           # queries per core
DPG = 64           # channels per group
OFF_K = 6
DS = 4             # downsample stride
OFF_SCALE = 4.0
NCORES = 8

# bias lookup table
HQ = 2.0 / 1023.0  # query grid step == table spacing
CQ = 1040.0        # index offset so c_j = CQ - vgsp1_j/HQ stays in [0.9, 1057]
LTAB = 1664        # table length (slabs need up to 128*8+127+511 = 1662)
NTT = 9            # tent kappa-tiles (c_j+1 < 1152)
NT = 8             # x position tiles for the kv gather
A_S = 1024.0 / 255.0   # d ppix / d j  (|offset| < 4 -> +-4*A_S slack)
A_O = 1023.0 / 255.0   # -d c / d j

# A&S 7.1.26 erf coefficients (|err| <= 1.5e-7)
ERF_P = 0.3275911
ERF_A = [0.254829592, -0.284496736, 1.421413741, -1.453152027, 1.061405429]

_CACHED = {}


def _s_window(t):
    """Static j-range where the kv-gather tent for position tile t can be
    nonzero: ppix_j in (128t-1, 128t+128), ppix = A_S*(j+off)-0.5, |off|<4."""
    jlo = max(0, int(np.floor((128 * t - 0.5) / A_S - 4)) - 1)
    jhi = min(NDS, int(np.ceil((128 * t + 128.5) / A_S + 4)) + 1)
    return jlo, jhi


def _o_window(t):
    """Static j-range where the bias tent for kappa tile t can be nonzero:
    c_j in (128t-1, 128t+128), c = CQ - A_O*(j+off), |off|<4."""
    jlo = max(0, int(np.floor((912 - 128 * t) / A_O - 4)) - 1)
    jhi = min(NDS, int(np.ceil((1041 - 128 * t) / A_O + 4)) + 1)
    return jlo, jhi


def _patch_act_tables():
    """Restrict activation-table selection to the single set that covers all
    ACT functions used by this kernel, so exactly one table load is emitted
    (runtime table swaps do not work in this environment)."""
    import concourse.hw_specs as hw_specs

    if getattr(bacc, "_deform_act_patch", False):
        return
    orig = hw_specs.get_activation_tables

    keep = "natural_log_exp_and_others"

    def patched(module_arch):
        tabs = orig(module_arch)
        keep_funcs = tabs[keep]
        out = {}
        for name, funcs in tabs.items():
            if name == keep:
                out[name] = funcs
            else:
                out[name] = funcs - keep_funcs
        return out

    bacc.get_activation_tables = patched
    bacc._deform_act_patch = True


def _erf_gelu(nc, sb, out_ap, x_ap, shape):
    """out = 0.5 * x * (1 + erf(x/sqrt(2))) via A&S 7.1.26 (no erf table).

    Writes (1 + erf(x/sqrt2)) * x  (WITHOUT the 0.5 -- folded into wproj).
    """
    P, Nf = shape
    sq = sb.tile([P, Nf], F32, name="gelu_sq", tag="gelu_sq")
    nc.scalar.activation(sq[:], x_ap, AF.Square)
    e = sb.tile([P, Nf], F32, name="gelu_e", tag="gelu_e")
    # e = exp(-x^2/2)
    nc.scalar.activation(e[:], sq[:], AF.Exp, scale=-0.5)
    ax = sb.tile([P, Nf], F32, name="gelu_ax", tag="gelu_ax")
    # |x|/sqrt(2) = max(x, -x) * (1/sqrt2): two steps
    nc.vector.scalar_tensor_tensor(ax[:], x_ap, -1.0, x_ap, ALU.mult, ALU.max)
    t = sb.tile([P, Nf], F32, name="gelu_t", tag="gelu_t")
    # t = 1 / (1 + p * |x| / sqrt2)
    nc.vector.tensor_scalar(t[:], ax[:], float(ERF_P / np.sqrt(2.0)), 1.0, ALU.mult, ALU.add)
    nc.vector.reciprocal(t[:], t[:])
    poly = sb.tile([P, Nf], F32, name="gelu_poly", tag="gelu_poly")
    # P(t) = a1 t + a2 t^2 + ... + a5 t^5 via (x + c)*t nested form
    nc.vector.tensor_scalar(poly[:], t[:], ERF_A[4], ERF_A[3], ALU.mult, ALU.add)
    nc.vector.tensor_tensor(poly[:], poly[:], t[:], ALU.mult)
    nc.vector.scalar_tensor_tensor(poly[:], poly[:], ERF_A[2], t[:], ALU.add, ALU.mult)
    nc.vector.scalar_tensor_tensor(poly[:], poly[:], ERF_A[1], t[:], ALU.add, ALU.mult)
    nc.vector.scalar_tensor_tensor(poly[:], poly[:], ERF_A[0], t[:], ALU.add, ALU.mult)
    # poly*e = 1 - erf(|x|/sqrt2)  =>  erfa = 1 - poly*e
    erfa = sb.tile([P, Nf], F32, name="gelu_erfa", tag="gelu_erfa")
    nc.vector.tensor_tensor(erfa[:], poly[:], e[:], ALU.mult)
    nc.vector.tensor_scalar(erfa[:], erfa[:], -1.0, 1.0, ALU.mult, ALU.add)
    # copysign: erf(x) = sign(x)*erfa
    sgn = sb.tile([P, Nf], U32, name="gelu_sgn", tag="gelu_sgn")
    nc.vector.tensor_scalar(sgn[:], x_ap.bitcast(U32), 0x80000000, None, ALU.bitwise_and)
    erfs = sb.tile([P, Nf], F32, name="gelu_erfs", tag="gelu_erfs")
    nc.vector.tensor_tensor(erfs[:].bitcast(U32), erfa[:].bitcast(U32), sgn[:], ALU.bitwise_or)
    # out = (1 + erf) * x    (0.5 folded into wproj)
    nc.vector.tensor_scalar(erfs[:], erfs[:], 1.0, None, ALU.add)
    nc.vector.tensor_tensor(out_ap, erfs[:], x_ap, ALU.mult)


def _tanh_rows(nc, sb, out_ap, x_ap, shape):
    """out = tanh(x) = sign(x) * (1 - 2/(exp(2*min(|x|,30))+1)) on small tiles."""
    P, Nf = shape
    ax = sb.tile([P, Nf], F32, name="th_ax", tag="th_ax")
    nc.vector.scalar_tensor_tensor(ax[:], x_ap, -1.0, x_ap, ALU.mult, ALU.max)
    nc.vector.tensor_scalar(ax[:], ax[:], 30.0, None, ALU.min)
    e = sb.tile([P, Nf], F32, name="th_e", tag="th_e")
    nc.scalar.activation(e[:], ax[:], AF.Exp, scale=2.0)
    nc.vector.tensor_scalar(e[:], e[:], 1.0, None, ALU.add)
    r = sb.tile([P, Nf], F32, name="th_r", tag="th_r")
    nc.vector.reciprocal(r[:], e[:])
    # tha = 1 - 2r
    nc.vector.tensor_scalar(r[:], r[:], -2.0, 1.0, ALU.mult, ALU.add)
    sgn = sb.tile([P, Nf], U32, name="th_sgn", tag="th_sgn")
    nc.vector.tensor_scalar(sgn[:], x_ap.bitcast(U32), 0x80000000, None, ALU.bitwise_and)
    nc.vector.tensor_tensor(out_ap.bitcast(U32), r[:].bitcast(U32), sgn[:], ALU.bitwise_or)


# packed-weights column layout ([128, PCK] f32, rows 0:64 used)
PK_WQT = 0          # wq[g].T               [64, 64]
PK_WKT = 64         # wk[g].T               [64, 64]
PK_WVT = 128        # wv[g].T               [64, 64]
PK_WOT = 192        # wo[:, group cols].T   [64, 256]
PK_WDW = 448        # depthwise conv taps   [64, 6]
PK_BDW = 454        # conv bias             [64, 1]
PK_WPJ = 455        # 0.5 * w_off_proj      [64, 1]
PK_WQTS = 456       # wq[g].T * DH^-0.5     [64, 64]
PK_EYE = 520        # identity              [64, 64]
PCK = 584


def build_nc():
    _patch_act_tables()
    nc = bacc.Bacc("TRN2", target_bir_lowering=False, debug=False, num_devices=NCORES)

    # ---- per-core DRAM inputs ----
    din = {}

    def dt_in(name, shape, dt=F32):
        din[name] = nc.dram_tensor(name, shape, dt, kind="ExternalInput")
        return din[name]

    dt_in("xg", [DPG, N], F32R)
    dt_in("xq", [DPG, QS], F32R)
    dt_in("xgT", [128, NT * DPG], F32R)
    dt_in("packed", [128, PCK])
    dt_in("slab", [128, 2 * NTT * QS], F32R)
    y_out = nc.dram_tensor("y", [DIM, QS], F32, kind="ExternalOutput")
    dbg = {}
    if DEBUG:
        for nm, shp in [("dbg_q", [DPG, N]), ("dbg_vgsp1", [1, NDS]),
                        ("dbg_kv", [DPG, NDS]), ("dbg_k", [DPG, NDS]),
                        ("dbg_v", [DPG, NDS]), ("dbg_qs", [DPG, QS]),
                        ("dbg_logit00", [128, QS]), ("dbg_avn", [DPG, QS])]:
            dbg[nm] = nc.dram_tensor(nm, shp, F32, kind="ExternalOutput")

    with tile.TileContext(nc) as tc:
        with (
            tc.tile_pool(name="const", bufs=1) as cst,
            tc.tile_pool(name="work", bufs=2) as wk,
            tc.tile_pool(name="rows", bufs=1) as rw,
            tc.tile_pool(name="persist", bufs=1) as pe_pool,
        ):
            # ---- input DMAs (issue order == queue order; slab last) ----
            packed = cst.tile([128, PCK], F32, name="packed", tag="packed")
            nc.sync.dma_start(packed[:], din["packed"].ap())
            xg = cst.tile([DPG, N], F32R, name="xg", tag="xg")
            nc.sync.dma_start(xg[:], din["xg"].ap())
            xq = cst.tile([DPG, QS], F32R, name="xq", tag="xq")
            nc.sync.dma_start(xq[:], din["xq"].ap())
            xgT = cst.tile([128, NT * DPG], F32R, name="xgT", tag="xgT")
            nc.sync.dma_start(xgT[:], din["xgT"].ap())
            slab = cst.tile([128, 2 * NTT * QS], F32R, name="slab", tag="slab")
            nc.sync.dma_start(slab[:], din["slab"].ap())

            # fp32r matmul inputs must be produced by a rounding instruction:
            # copy the weight slices into one fp32r tile
            wts = cst.tile([DPG, PK_WQTS + DPG], F32R, name="wts", tag="wts")
            nc.vector.tensor_copy(wts[:], packed[0:DPG, 0:PK_WQTS + DPG])

            def W(col, width):
                return wts[:, col:col + width]

            eye64 = packed[0:DPG, PK_EYE:PK_EYE + DPG]

            ones_col = cst.tile([128, 1], F32, name="ones", tag="ones")
            nc.gpsimd.memset(ones_col[:], 1.0)
            # dummy activation: triggers the (single) ACT table load at t=0 so
            # it overlaps the input DMAs instead of sitting in the offsets chain
            warm = cst.tile([128, 1], F32, name="warm", tag="warm")
            nc.scalar.activation(warm[:], ones_col[:], AF.Relu)
            ones_colr = cst.tile([128, 1], F32R, name="onesr", tag="onesr")
            nc.vector.tensor_copy(ones_colr[:], ones_col[:])
            iotac = cst.tile([128, 1], I32, name="iotac", tag="iotac")
            nc.gpsimd.iota(iotac[:], pattern=[[0, 1]], base=0, channel_multiplier=1)
            iotacf = cst.tile([128, 1], F32, name="iotacf", tag="iotacf")
            nc.vector.tensor_copy(iotacf[:], iotac[:])

            # tent matrices (zeroed once; only static j-windows written later)
            S_all = pe_pool.tile([128, NT * NDS], F32R, name="S_all", tag="S_all")
            nc.gpsimd.memset(S_all[:].bitcast(F32), 0.0)
            OHT_all = pe_pool.tile([128, NTT * NDS], F32R, name="OHT_all", tag="OHT_all")
            nc.gpsimd.memset(OHT_all[:].bitcast(F32), 0.0)

            # persistent SBUF tiles that cross phase boundaries
            k_sb = pe_pool.tile([DPG, NDS], F32R, name="k_sb", tag="k_sb")
            qs_sb = pe_pool.tile([DPG, QS], F32R, name="qs_sb", tag="qs_sb")
            vT = [pe_pool.tile([128, DPG], F32R, name=f"vT{H}", tag=f"vT{H}") for H in range(2)]
            avn = pe_pool.tile([DPG, QS], F32R, name="avn", tag="avn")

            # ============ phase A: q, qs, offsets ============
            with tc.tile_pool(name="psA", bufs=2, space="PSUM") as psA:
                q_pad = pe_pool.tile([DPG, N + 2], F32, name="q_pad", tag="q_pad")
                nc.gpsimd.memset(q_pad[:, 0:1], 0.0)
                nc.gpsimd.memset(q_pad[:, N + 1:N + 2], 0.0)
                for h in range(2):
                    pq = psA.tile([DPG, QS], F32, name="pA512", tag="pA512")
                    nc.tensor.matmul(pq[:], W(PK_WQT, DPG), xg[:, h * QS:(h + 1) * QS])
                    nc.scalar.copy(q_pad[:, 1 + h * QS:1 + (h + 1) * QS], pq[:])

                # qs for this core's query half (from xq; scale folded in wqTs)
                pqs = psA.tile([DPG, QS], F32, name="pA512", tag="pA512")
                nc.tensor.matmul(pqs[:], W(PK_WQTS, DPG), xq[:])
                nc.scalar.copy(qs_sb[:], pqs[:])

                # depthwise strided conv (6 taps)
                acc = wk.tile([DPG, NDS], F32, name="conv_acc", tag="conv_acc")
                nc.vector.tensor_scalar(
                    acc[:], q_pad[:, 0:N - 3:DS], packed[0:DPG, PK_WDW:PK_WDW + 1],
                    packed[0:DPG, PK_BDW:PK_BDW + 1], ALU.mult, ALU.add)
                for kk in range(1, OFF_K):
                    nc.vector.scalar_tensor_tensor(
                        acc[:], q_pad[:, kk:kk + N - 3:DS],
                        packed[0:DPG, PK_WDW + kk:PK_WDW + kk + 1], acc[:],
                        ALU.mult, ALU.add)

                if DEBUG:
                    nc.sync.dma_start(dbg["dbg_q"].ap(), q_pad[:, 1:N + 1])
                    nc.sync.dma_start(dbg["dbg_qs"].ap(), qs_sb[:].bitcast(F32))
                gl = wk.tile([DPG, NDS], F32R, name="gelu_out", tag="gelu_out")
                _erf_gelu(nc, wk, gl[:], acc[:], [DPG, NDS])

                # proj row: [1, NDS] = sum_c 0.5*wproj[c] * gl[c, :]
                pproj = psA.tile([1, NDS], F32, name="pproj", tag="pproj")
                nc.tensor.matmul(pproj[:], W(PK_WPJ, 1), gl[:])
                proj_sb = rw.tile([1, NDS], F32, name="proj_sb", tag="proj_sb")
                nc.vector.tensor_copy(proj_sb[:], pproj[:])
                th = rw.tile([1, NDS], F32, name="th", tag="th")
                _tanh_rows(nc, rw, th[:], proj_sb[:], [1, NDS])

                # vgrid = j + 4*tanh ; vgsp1 = vgrid*2/255
                iotaj = rw.tile([1, NDS], I32, name="iotaj", tag="iotaj")
                nc.gpsimd.iota(iotaj[:], pattern=[[1, NDS]], base=0, channel_multiplier=0)
                iotajf = rw.tile([1, NDS], F32, name="iotajf", tag="iotajf")
                nc.vector.tensor_copy(iotajf[:], iotaj[:])
                vgsp1 = rw.tile([1, NDS], F32, name="vgsp1", tag="vgsp1")
                nc.vector.scalar_tensor_tensor(vgsp1[:], th[:], OFF_SCALE, iotajf[:], ALU.mult, ALU.add)
                nc.vector.tensor_scalar(vgsp1[:], vgsp1[:], float(2.0 / (NDS - 1)), None, ALU.mult)
                # ppix = vgsp1*512 - 0.5 ; c = CQ - vgsp1/HQ
                ppix = rw.tile([1, NDS], F32, name="ppix", tag="ppix")
                nc.vector.tensor_scalar(ppix[:], vgsp1[:], float(N / 2.0), -0.5, ALU.mult, ALU.add)
                c_row = rw.tile([1, NDS], F32, name="c_row", tag="c_row")
                nc.vector.tensor_scalar(c_row[:], vgsp1[:], float(-1.0 / HQ), CQ, ALU.mult, ALU.add)
                if DEBUG:
                    nc.sync.dma_start(dbg["dbg_vgsp1"].ap(), vgsp1[:])

                ppix_bc = pe_pool.tile([128, NDS], F32, name="ppix_bc", tag="ppix_bc")
                nc.gpsimd.partition_broadcast(ppix_bc[:], ppix[:])
                c_bc = pe_pool.tile([128, NDS], F32, name="c_bc", tag="c_bc")
                nc.gpsimd.partition_broadcast(c_bc[:], c_row[:])

                # ---- phase B: kv gather via tents, k, v, vT ----
                pkv = psA.tile([DPG, NDS], F32, name="pA256", tag="pA256")
                for t in range(NT):
                    jlo, jhi = _s_window(t)
                    w = jhi - jlo
                    wS = wk.tile([128, 48], F32, name="wS", tag="wS")
                    # w = (ppix - kappa'): tent = relu(1 - |w|)
                    nc.vector.tensor_scalar(
                        wS[:, 0:w], ppix_bc[:, jlo:jhi], iotacf[:], float(128 * t),
                        ALU.subtract, ALU.subtract)
                    nc.vector.scalar_tensor_tensor(
                        wS[:, 0:w], wS[:, 0:w], -1.0, wS[:, 0:w], ALU.mult, ALU.min)
                    nc.scalar.activation(
                        S_all[:, NDS * t + jlo:NDS * t + jhi], wS[:, 0:w], AF.Relu, bias=1.0)
                    nc.tensor.matmul(pkv[:], xgT[:, DPG * t:DPG * (t + 1)],
                                     S_all[:, NDS * t:NDS * (t + 1)],
                                     start=(t == 0), stop=(t == NT - 1))
                kv = wk.tile([DPG, NDS], F32R, name="kv", tag="kv")
                nc.scalar.copy(kv[:], pkv[:])
                if DEBUG:
                    nc.sync.dma_start(dbg["dbg_kv"].ap(), kv[:].bitcast(F32))

                pk = psA.tile([DPG, NDS], F32, name="pA256", tag="pA256")
                nc.tensor.matmul(pk[:], W(PK_WKT, DPG), kv[:])
                nc.scalar.copy(k_sb[:], pk[:])
                pv = psA.tile([DPG, NDS], F32, name="pA256", tag="pA256")
                nc.tensor.matmul(pv[:], W(PK_WVT, DPG), kv[:])
                v_sb = wk.tile([DPG, NDS], F32, name="v_sb", tag="v_sb")
                nc.scalar.copy(v_sb[:], pv[:])
                if DEBUG:
                    nc.sync.dma_start(dbg["dbg_k"].ap(), k_sb[:].bitcast(F32))
                    nc.sync.dma_start(dbg["dbg_v"].ap(), v_sb[:])

                for H in range(2):
                    pt = psA.tile([128, DPG], F32, name="ptp", tag="ptp")
                    nc.tensor.transpose(pt[:], v_sb[:, H * 128:(H + 1) * 128], eye64)
                    nc.scalar.copy(vT[H][:], pt[:])

                # ---- phase C: bias interpolation tents ----
                for t in range(NTT):
                    jlo, jhi = _o_window(t)
                    if jhi <= jlo:
                        continue
                    w = jhi - jlo
                    wT = wk.tile([128, 48], F32, name="wT", tag="wT")
                    nc.vector.tensor_scalar(
                        wT[:, 0:w], c_bc[:, jlo:jhi], iotacf[:], float(128 * t),
                        ALU.subtract, ALU.subtract)
                    nc.vector.scalar_tensor_tensor(
                        wT[:, 0:w], wT[:, 0:w], -1.0, wT[:, 0:w], ALU.mult, ALU.min)
                    nc.scalar.activation(
                        OHT_all[:, NDS * t + jlo:NDS * t + jhi], wT[:, 0:w], AF.Relu, bias=1.0)

            # ============ phase D: attention (bias accumulated in PSUM) ============
            with (
                tc.tile_pool(name="psE", bufs=4, space="PSUM") as psE,
                tc.tile_pool(name="psE1", bufs=2, space="PSUM") as psE1,
            ):
                for h in range(2):
                    expT = []
                    for H in range(2):
                        psim = psE.tile([128, QS], F32, name="psim", tag="psim")
                        nc.tensor.matmul(
                            psim[:], k_sb[32 * h:32 * (h + 1), H * 128:(H + 1) * 128],
                            qs_sb[32 * h:32 * (h + 1), :], start=True, stop=False)
                        for t in range(NTT):
                            nc.tensor.matmul(
                                psim[:],
                                OHT_all[:, NDS * t + 128 * H:NDS * t + 128 * (H + 1)],
                                slab[:, (h * NTT + t) * QS:(h * NTT + t + 1) * QS],
                                start=False, stop=(t == NTT - 1))
                        if DEBUG and h == 0 and H == 0:
                            lg = wk.tile([128, QS], F32, name="lg", tag="lg")
                            nc.vector.tensor_copy(lg[:], psim[:])
                            nc.sync.dma_start(dbg["dbg_logit00"].ap(), lg[:])
                        et = wk.tile([128, QS], F32R, name="expT", tag="expT")
                        nc.scalar.activation(et[:], psim[:], AF.Exp)
                        expT.append(et)

                    # sums over j via ones-matmul, then reciprocal
                    psum_s = psE1.tile([1, QS], F32, name="psum_s", tag="psum_s")
                    for H in range(2):
                        nc.tensor.matmul(psum_s[:], ones_colr[:], expT[H][:],
                                         start=(H == 0), stop=(H == 1))
                    rs = rw.tile([1, QS], F32, name="rs", tag="rs")
                    nc.vector.reciprocal(rs[:], psum_s[:])
                    rsb = wk.tile([32, QS], F32, name="rsb", tag="rsb")
                    nc.gpsimd.partition_broadcast(rsb[:], rs[:])

                    pav = psE1.tile([32, QS], F32, name="pav", tag="pav")
                    for H in range(2):
                        nc.tensor.matmul(pav[:], vT[H][:, 32 * h:32 * (h + 1)], expT[H][:],
                                         start=(H == 0), stop=(H == 1))
                    nc.vector.tensor_tensor(avn[32 * h:32 * (h + 1), :], pav[:], rsb[:], ALU.mult)

                if DEBUG:
                    nc.sync.dma_start(dbg["dbg_avn"].ap(), avn[:].bitcast(F32))
                for m in range(2):
                    py = psE.tile([128, QS], F32, name="py", tag="psim")
                    nc.tensor.matmul(py[:], W(PK_WOT + m * 128, 128), avn[:])
                    y_sb = wk.tile([128, QS], F32, name="y_sb", tag="y_sb")
                    nc.scalar.copy(y_sb[:], py[:])
                    nc.sync.dma_start(y_out.ap()[m * 128:(m + 1) * 128, :], y_sb[:])

    nc.compile()
    return nc


def _shard_inputs(inputs):
    """Build the 8 per-core input maps from the full inputs."""
    x = np.ascontiguousarray(inputs["x"][0])               # [256, 1024]
    wq, wk, wv = inputs["wq"], inputs["wk"], inputs["wv"]  # [4, 64, 64]
    wo = inputs["wo"]                                      # [256, 256]
    w_off_dw = inputs["w_off_dw"][:, 0, :]                 # [64, 6]
    b_off_dw = inputs["b_off_dw"]                          # [64]
    w_off_proj = inputs["w_off_proj"]                      # [64]
    w1 = inputs["cpb_w1"][:, 0].astype(np.float64)         # [64]
    b1 = inputs["cpb_b1"].astype(np.float64)
    w2 = inputs["cpb_w2"].astype(np.float64)
    b2 = inputs["cpb_b2"].astype(np.float64)
    w3 = inputs["cpb_w3"].astype(np.float64)               # [2, 64]
    b3 = inputs["cpb_b3"].astype(np.float64)

    f = np.float32

    # bias lookup tables + Hankel slabs, one per query-half (weights-only)
    slabs = {}
    for qh in range(2):
        kk = np.arange(LTAB, dtype=np.float64)
        d = HQ * (kk - CQ + QS * qh)
        pos = np.sign(d) * np.log1p(np.abs(d))
        h1 = np.maximum(pos[:, None] * w1[None, :] + b1, 0.0)
        h2 = np.maximum(h1 @ w2.T + b2, 0.0)
        T = (h2 @ w3.T + b3).astype(f)                     # [LTAB, 2]
        sl = np.zeros((128, 2 * NTT * QS), f)
        for o in range(2):
            sw = np.lib.stride_tricks.sliding_window_view(T[:, o], QS)
            for t in range(NTT):
                sl[:, (o * NTT + t) * QS:(o * NTT + t + 1) * QS] = sw[128 * t:128 * t + 128]
        slabs[qh] = sl

    base_packed = np.zeros((128, PCK), f)
    base_packed[0:DPG, PK_WDW:PK_WDW + OFF_K] = w_off_dw
    base_packed[0:DPG, PK_BDW] = b_off_dw
    base_packed[0:DPG, PK_WPJ] = 0.5 * w_off_proj
    base_packed[0:DPG, PK_EYE:PK_EYE + DPG] = np.eye(DPG, dtype=f)

    in_maps = []
    for c in range(NCORES):
        g, qh = c // 2, c % 2
        xg = np.ascontiguousarray(x[DPG * g:DPG * (g + 1)], dtype=f)
        xgT = np.zeros((128, NT * DPG), f)
        for t in range(NT):
            xgT[:, DPG * t:DPG * (t + 1)] = xg[:, 128 * t:128 * (t + 1)].T
        pk = base_packed.copy()
        pk[0:DPG, PK_WQT:PK_WQT + DPG] = wq[g].T
        pk[0:DPG, PK_WKT:PK_WKT + DPG] = wk[g].T
        pk[0:DPG, PK_WVT:PK_WVT + DPG] = wv[g].T
        pk[0:DPG, PK_WOT:PK_WOT + DIM] = wo[:, DPG * g:DPG * (g + 1)].T
        pk[0:DPG, PK_WQTS:PK_WQTS + DPG] = wq[g].T * f(DH) ** f(-0.5)
        m = {
            "xg": xg,
            "xq": np.ascontiguousarray(xg[:, QS * qh:QS * (qh + 1)]),
            "xgT": xgT,
            "packed": pk,
            "slab": slabs[qh],
        }
        in_maps.append(m)
    return in_maps


def kernel(**inputs):
    if "nc" not in _CACHED:
        _CACHED["nc"] = build_nc()
    nc = _CACHED["nc"]
    in_maps = _shard_inputs(inputs)
    res = bass_utils.run_bass_kernel_spmd(nc, in_maps, core_ids=list(range(NCORES)))
    ys = [res.results[c]["y"] for c in range(NCORES)]
    bo = inputs["bo"]
    out = np.zeros((1, DIM, N), np.float32)
    for qh in range(2):
        acc = np.zeros((DIM, QS), np.float64)
        for g in range(G):
            acc += ys[2 * g + qh]
        out[0, :, QS * qh:QS * (qh + 1)] = (acc + bo.astype(np.float64)[:, None]).astype(np.float32)
    return out
